# revision 6
# baseline (speedup 1.0000x reference)
"""AxialAttention TRN2 kernel.

Sharding: 8 cores = 4 batches x 2 head-groups (4 heads each). Each core:
  phase 1: qkv projection (fp32r), x-stationary -> s-major qkv [S, 768] bf16 in DRAM
  phase 2: row + col axial attention (bf16 matmuls, softmax without max-subtraction;
           per-query sums via ones-matmul broadcast; normalization fused into drains)
  phase 3: output projection (bf16) of row+col sum, bias on group-0 cores only
Host: per-batch sum of the two head-group partial outputs.
"""

import numpy as np
import ml_dtypes
from contextlib import ExitStack

import concourse.bass as bass
import concourse.bacc as bacc
import concourse.tile as tile
from concourse import mybir
from concourse.bass_utils import run_bass_kernel_spmd

C = 512          # channels
H = 128          # height
W = 128          # width
S = H * W        # 16384 pixels
NH = 8           # total heads
D = 64           # head dim
NHC = 4          # heads per core
GC = NHC * D     # 256 group channels (q or k or v)
QKV = 3 * GC     # 768 projected channels per core
CT = C // 128    # 4 contraction tiles
ST = S // 128    # 128 spatial tiles
SCALE = 1.0 / np.sqrt(D)

F32 = mybir.dt.float32
F32R = mybir.dt.float32r
BF16 = mybir.dt.bfloat16
EXP = mybir.ActivationFunctionType.Exp
IDENT = mybir.ActivationFunctionType.Identity
ADD = mybir.AluOpType.add
MULT = mybir.AluOpType.mult

_CACHED_NC = None


def build_nc(debug_dump=False):
    nc = bacc.Bacc()
    x_in = nc.dram_tensor("x", [C, S], F32R, kind="ExternalInput")
    wqkvT = nc.dram_tensor("wqkvT", [C, QKV], F32R, kind="ExternalInput")
    bqkv = nc.dram_tensor("bqkv", [1, QKV], F32, kind="ExternalInput")
    woutT = nc.dram_tensor("woutT", [GC, C], BF16, kind="ExternalInput")
    bout = nc.dram_tensor("bout", [128, CT], F32, kind="ExternalInput")
    out = nc.dram_tensor("out", [C, S], F32, kind="ExternalOutput")
    if debug_dump:
        dbg_qkv = nc.dram_tensor("dbg_qkv", [S, QKV], BF16, kind="ExternalOutput")
        dbg_O = nc.dram_tensor("dbg_O", [2, 128, S], BF16, kind="ExternalOutput")

    with tile.TileContext(nc) as tc, ExitStack() as ctx:
        persist = ctx.enter_context(tc.tile_pool(name="persist", bufs=1))
        dram = ctx.enter_context(tc.tile_pool(name="dram", bufs=1, space="DRAM"))

        # --- persistent tiles ---
        w_sb = persist.tile([128, CT, QKV], F32R, tag="w_sb")
        nc.scalar.dma_start(
            out=w_sb, in_=wqkvT.ap().rearrange("(t p) o -> p t o", p=128)
        )
        bias_sb = persist.tile([128, QKV], F32, tag="bias_sb")
        bq_ap = bqkv.ap()
        nc.scalar.dma_start(
            out=bias_sb,
            in_=bass.AP(tensor=bq_ap.tensor, offset=0, ap=[[0, 128], [1, QKV]]),
        )
        wout_sb = persist.tile([128, 2, C], BF16, tag="wout_sb")
        nc.scalar.dma_start(
            out=wout_sb, in_=woutT.ap().rearrange("(t p) o -> p t o", p=128)
        )
        boutv = persist.tile([128, CT], F32, tag="boutv")
        nc.scalar.dma_start(out=boutv, in_=bout.ap())
        ones_sb = persist.tile([128, 128], BF16, tag="ones_sb")
        nc.vector.memset(ones_sb, 1.0)

        O_sb = [
            persist.tile([128, S], BF16, tag=f"O{i}", name=f"O{i}") for i in range(2)
        ]

        qkvB = dram.tile([S, QKV], BF16)

        # ---------- phase 1: qkv projection (x stationary, s-major out) ----------
        x_r = x_in.ap().rearrange("(t p) s -> p t s", p=128)
        with (
            tc.tile_pool(name="p1x", bufs=3) as xpool,
            tc.tile_pool(name="p1ps", bufs=2, space="PSUM") as pspool,
            tc.tile_pool(name="p1o", bufs=3) as opool,
        ):
            for sg in range(ST // 4):  # groups of 4 s-tiles
                xg = xpool.tile([128, CT, 512], F32R)
                nc.scalar.dma_start(out=xg, in_=x_r[:, :, sg * 512 : (sg + 1) * 512])
                for i in range(4):
                    st = sg * 4 + i
                    ps = pspool.tile([128, QKV], F32)
                    for ct in range(CT):
                        lhsT = xg[:, ct, i * 128 : (i + 1) * 128]
                        nc.tensor.matmul(
                            out=ps[:, 0:512],
                            lhsT=lhsT,
                            rhs=w_sb[:, ct, 0:512],
                            start=(ct == 0),
                            stop=(ct == CT - 1),
                        )
                        nc.tensor.matmul(
                            out=ps[:, 512:QKV],
                            lhsT=lhsT,
                            rhs=w_sb[:, ct, 512:QKV],
                            start=(ct == 0),
                            stop=(ct == CT - 1),
                        )
                    qt = opool.tile([128, QKV], BF16)
                    nc.vector.tensor_tensor(out=qt, in0=ps, in1=bias_sb, op=ADD)
                    nc.scalar.dma_start(
                        out=qkvB[st * 128 : (st + 1) * 128, :], in_=qt
                    )

        # ---------- phase 2: axial attention ----------
        qkv_row = qkvB[:]  # [S, QKV]: row tile t = rows t*128..
        qkv_col = qkvB[:].rearrange("(h w) o -> w h o", w=W)  # col tile t = [:, t, :]
        with (
            tc.tile_pool(name="a_qt", bufs=6) as qtpool,
            tc.tile_pool(name="a_kt", bufs=6) as ktpool,
            tc.tile_pool(name="a_vt", bufs=6) as vtpool,
            tc.tile_pool(name="a_p", bufs=3) as ppool,
            tc.tile_pool(name="a_pn", bufs=3) as pnpool,
            tc.tile_pool(name="a_rz", bufs=3) as rzpool,
            tc.tile_pool(name="a_psS", bufs=2, space="PSUM") as psumS,
            tc.tile_pool(name="a_psZ", bufs=2, space="PSUM") as psumZ,
            tc.tile_pool(name="a_psO", bufs=2, space="PSUM") as psumO,
        ):
            for branch in range(2):  # 0 = row (writes O), 1 = col (adds into O)
                for hp in range(2):  # head pair
                    qcol = hp * 128
                    kcol = 256 + hp * 128
                    vcol = 512 + hp * 128
                    for tg in range(ST // 4):  # chunks of 4 attention tiles
                        qts, kts, vts = [], [], []
                        for i in range(4):
                            t = tg * 4 + i
                            if branch == 0:
                                rows = qkv_row[t * 128 : (t + 1) * 128, :]
                            else:
                                rows = qkv_col[t, :, :]
                            qt2 = qtpool.tile([128, 128], BF16)
                            nc.sync.dma_start_transpose(
                                out=qt2, in_=rows[:, qcol : qcol + 128]
                            )
                            kt2 = ktpool.tile([128, 128], BF16)
                            nc.sync.dma_start_transpose(
                                out=kt2, in_=rows[:, kcol : kcol + 128]
                            )
                            vt = vtpool.tile([128, 128], BF16)
                            nc.scalar.dma_start(
                                out=vt, in_=rows[:, vcol : vcol + 128]
                            )
                            qts.append(qt2)
                            kts.append(kt2)
                            vts.append(vt)
                        for hl in range(2):  # head within pair
                            r0, r1 = hl * 64, (hl + 1) * 64
                            psS = psumS.tile([128, 512], F32)
                            for i in range(4):
                                nc.tensor.matmul(
                                    out=psS[:, i * 128 : (i + 1) * 128],
                                    lhsT=kts[i][r0:r1, :],
                                    rhs=qts[i][r0:r1, :],
                                    start=True,
                                    stop=True,
                                )
                            pch = ppool.tile([128, 512], BF16)
                            nc.scalar.activation(
                                out=pch, in_=psS, func=EXP, scale=float(SCALE)
                            )
                            psZ = psumZ.tile([128, 512], F32)
                            nc.tensor.matmul(
                                out=psZ, lhsT=ones_sb, rhs=pch, start=True, stop=True
                            )
                            rz = rzpool.tile([128, 512], F32)
                            nc.vector.reciprocal_approx_fast(out=rz, in_=psZ)
                            if branch == 1:
                                pn = pnpool.tile([128, 512], BF16)
                                nc.vector.tensor_tensor(
                                    out=pn, in0=pch, in1=rz, op=MULT
                                )
                                puse = pn
                            else:
                                puse = pch
                            psO = psumO.tile([64, 512], F32)
                            for i in range(4):
                                nc.tensor.matmul(
                                    out=psO[:, i * 128 : (i + 1) * 128],
                                    lhsT=vts[i][:, r0:r1],
                                    rhs=puse[:, i * 128 : (i + 1) * 128],
                                    start=True,
                                    stop=True,
                                )
                            if branch == 0:
                                nc.vector.tensor_tensor(
                                    out=O_sb[hp][r0:r1, tg * 512 : (tg + 1) * 512],
                                    in0=psO,
                                    in1=rz[0:64, :],
                                    op=MULT,
                                )
                            else:
                                dst = O_sb[hp][r0:r1, :].rearrange(
                                    "p (h w) -> p h w", w=W
                                )[:, :, tg * 4 : (tg + 1) * 4]
                                nc.vector.tensor_tensor(
                                    out=dst,
                                    in0=psO.rearrange("p (w h) -> p h w", w=4),
                                    in1=dst,
                                    op=ADD,
                                )

        if debug_dump:
            nc.scalar.dma_start(out=dbg_qkv[:], in_=qkvB[:])
            for i in range(2):
                nc.scalar.dma_start(out=dbg_O.ap()[i], in_=O_sb[i])

        # ---------- phase 3: output projection ----------
        with (
            tc.tile_pool(name="f_ps", bufs=2, space="PSUM") as psumF,
            tc.tile_pool(name="f_o", bufs=3) as fpool,
        ):
            for ch in range(S // 512):
                for ot in range(CT):
                    psF = psumF.tile([128, 512], F32)
                    for hp in range(2):
                        nc.tensor.matmul(
                            out=psF,
                            lhsT=wout_sb[:, hp, ot * 128 : (ot + 1) * 128],
                            rhs=O_sb[hp][:, ch * 512 : (ch + 1) * 512],
                            start=(hp == 0),
                            stop=(hp == 1),
                        )
                    of = fpool.tile([128, 512], F32)
                    nc.scalar.activation(
                        out=of,
                        in_=psF,
                        func=IDENT,
                        bias=boutv[:, ot : ot + 1],
                        scale=1.0,
                    )
                    nc.scalar.dma_start(
                        out=out.ap()[
                            ot * 128 : (ot + 1) * 128, ch * 512 : (ch + 1) * 512
                        ],
                        in_=of,
                    )

    nc.finalize()
    return nc


def get_nc():
    global _CACHED_NC
    if _CACHED_NC is None:
        _CACHED_NC = build_nc()
    return _CACHED_NC


def make_in_maps(x, Wqkv, bqkv, Wout, bout):
    """Per-core input dicts: core c = (b, g) with b = c // 2, g = c % 2."""
    B = x.shape[0]
    in_maps = []
    for c in range(8):
        b, g = c // 2, c % 2
        sel = slice(256 * g, 256 * (g + 1))
        wsel = np.concatenate(
            [Wqkv[sel, :], Wqkv[512 + 256 * g : 512 + 256 * (g + 1), :],
             Wqkv[1024 + 256 * g : 1024 + 256 * (g + 1), :]], axis=0
        )  # [768, 512]
        bsel = np.concatenate(
            [bqkv[sel], bqkv[512 + 256 * g : 512 + 256 * (g + 1)],
             bqkv[1024 + 256 * g : 1024 + 256 * (g + 1)]]
        )  # [768]
        woutT = np.ascontiguousarray(Wout[:, sel].T)  # [256, 512]
        in_maps.append(
            {
                "x": np.ascontiguousarray(x[b].reshape(C, S)),
                "wqkvT": np.ascontiguousarray(wsel.T),
                "bqkv": bsel.reshape(1, QKV).copy(),
                "woutT": woutT.astype(ml_dtypes.bfloat16),
                "bout": (
                    np.ascontiguousarray(bout.reshape(CT, 128).T)
                    if g == 0
                    else np.zeros((128, CT), np.float32)
                ),
            }
        )
    return in_maps


def kernel(x, Wqkv, bqkv, Wout, bout):
    x = np.asarray(x, dtype=np.float32)
    Wqkv = np.asarray(Wqkv, dtype=np.float32)
    bqkv = np.asarray(bqkv, dtype=np.float32)
    Wout = np.asarray(Wout, dtype=np.float32)
    bout = np.asarray(bout, dtype=np.float32)

    nc = get_nc()
    in_maps = make_in_maps(x, Wqkv, bqkv, Wout, bout)
    res = run_bass_kernel_spmd(nc, in_maps, core_ids=list(range(8)))
    B = x.shape[0]
    out = np.empty((B, C, H, W), dtype=np.float32)
    for b in range(B):
        acc = res.results[2 * b]["out"] + res.results[2 * b + 1]["out"]
        out[b] = acc.reshape(C, H, W)
    return out
